# revision 13
# baseline (speedup 1.0000x reference)
"""Trainium2 Bass kernel for the autoregressive 2-layer LSTM (nn_ArLSTM).

Strategy (phase A): data-parallel over batch. B=64 is sharded 8 ways (8
sequences per core); each core runs the full T=512 sequential scan locally
with no cross-core communication.

Algebraic restructuring vs the reference:
  - x_main @ Wih0_m is composed:  Wih0[:, :H] @ Wp  is one [4H, D_IN] matrix,
    so the per-step input term  pre0[t] = (Wih0_m@Wp) @ main_t + bih0 + bhh0
    is a parallel GEMM over all (b, t), done on-device before the scan.
  - emb[prev] enters only through Wih0[:, H:] @ emb[prev].  E0 = Wih0_e@emb.T
    is a [4H, 11] matrix; the per-step term is E0 @ onehot(prev), a K=11
    matmul.  onehot is built from the logits with a max-compare (no gather).
  - The head's phys term  W1[:, H:] @ ph_t + b1  is precomputed per (b, t).
  - All matmuls run in fp16 (1 cycle/row on PE) with fp32 PSUM accumulation;
    cell state c and all element-wise math stay fp32.  Measured end-to-end
    rel_err vs the fp32 reference ~6.5e-3 (argmax feedback is contractive).

Gate-bank layout: gates [4H, B_local] live in one PSUM bank [128, 16*BL]
where column-block m holds gate rows 128m..128m+127.  With PyTorch gate
order (i, f, g, o) the quarters are column ranges, so the whole LSTM cell
is element-wise ops on [128, 4*BL] slices.
"""

import os
import numpy as np

import concourse.bass as bass
import concourse.tile as tile
from concourse import bacc, mybir
from concourse.bass import ds, ts
from concourse.bass_utils import run_bass_kernel_spmd

F16 = mybir.dt.float16
F32 = mybir.dt.float32

B, T, D_IN, D_PHYS, H, C = 64, 512, 256, 32, 512, 11
NCORES = 8
BL = B // NCORES          # 8 sequences per core
G = 4 * H                 # 2048 gate rows
MT = G // 128             # 16 gate m-tiles
TOK = None                # set per build (t_steps * BL)


def _tile_stationary(wt: np.ndarray) -> np.ndarray:
    """[K, M] -> [128, (K/128)*(M/128)*128] fp16 stationary-tile layout.

    Free index = ((k_chunk*MT_loc + m_tile)*128 + col)."""
    K, M = wt.shape
    kc, mt = K // 128, M // 128
    return (
        wt.reshape(kc, 128, mt, 128).transpose(1, 0, 2, 3).reshape(128, kc * mt * 128)
    ).astype(np.float16)


def build(t_steps: int, reps: int = 1):
    tok = t_steps * BL
    nc = bacc.Bacc(None, target_bir_lowering=False, debug=False)

    # ---- DRAM parameters (per-core inputs) ----
    mainT = nc.declare_dram_parameter("mainT", [D_IN + 1, tok], F16, isOutput=False)    # aug ones row
    physT = nc.declare_dram_parameter("physT", [D_PHYS + 1, tok], F16, isOutput=False)  # aug ones row
    McompT = nc.declare_dram_parameter("McompT", [128, 2 * MT * 128], F16, isOutput=False)
    McompB = nc.declare_dram_parameter("McompB", [1, MT * 128], F16, isOutput=False)    # bias row
    W1pT = nc.declare_dram_parameter("W1pT", [D_PHYS + 1, 4 * 128], F16, isOutput=False)
    Whh0T = nc.declare_dram_parameter("Whh0T", [128, 4 * MT * 128], F16, isOutput=False)
    Wih1T = nc.declare_dram_parameter("Wih1T", [128, 4 * MT * 128], F16, isOutput=False)
    Whh1T = nc.declare_dram_parameter("Whh1T", [128, 4 * MT * 128], F16, isOutput=False)
    W1hT = nc.declare_dram_parameter("W1hT", [128, 4 * 4 * 128], F16, isOutput=False)
    W2T = nc.declare_dram_parameter("W2T", [128, 4 * C], F16, isOutput=False)
    E0T = nc.declare_dram_parameter("E0T", [C, MT * 128], F16, isOutput=False)
    BIAS1 = nc.declare_dram_parameter("BIAS1", [128, MT * BL], F16, isOutput=False)
    IDT = nc.declare_dram_parameter("IDT", [128, 128], F16, isOutput=False)
    b2col = nc.declare_dram_parameter("b2col", [C, 1], F32, isOutput=False)
    b2r = nc.declare_dram_parameter("b2r", [1, C], F16, isOutput=False)

    pre0_d = nc.dram_tensor("pre0_d", [128, t_steps, MT * BL], F16)
    preh_d = nc.dram_tensor("preh_d", [128, t_steps, 4 * BL], F16)
    out_d = nc.declare_dram_parameter("out_logits", [t_steps, C, BL], F32, isOutput=True)

    AF = mybir.ActivationFunctionType

    with tile.TileContext(nc) as tc:
        with tc.tile_pool(name="wpool", bufs=1) as wp:
            # persistent weights in SBUF
            whh0 = wp.tile([128, 4 * MT * 128], F16)
            wih1 = wp.tile([128, 4 * MT * 128], F16)
            whh1 = wp.tile([128, 4 * MT * 128], F16)
            w1h = wp.tile([128, 4 * 4 * 128], F16)
            w2 = wp.tile([128, 4 * C], F16)
            e0 = wp.tile([C, MT * 128], F16)
            bias1 = wp.tile([128, MT * BL], F16)
            idt = wp.tile([128, 128], F16)
            b2s = wp.tile([C, 1], F32)
            b2row = wp.tile([1, C], F16)
            onesrow = wp.tile([1, BL], F16)
            nc.sync.dma_start(out=whh0[:], in_=Whh0T[:])
            nc.sync.dma_start(out=wih1[:], in_=Wih1T[:])
            nc.sync.dma_start(out=whh1[:], in_=Whh1T[:])
            nc.sync.dma_start(out=w1h[:], in_=W1hT[:])
            nc.sync.dma_start(out=w2[:], in_=W2T[:])
            nc.sync.dma_start(out=e0[:], in_=E0T[:])
            nc.sync.dma_start(out=bias1[:], in_=BIAS1[:])
            nc.sync.dma_start(out=idt[:], in_=IDT[:])
            nc.sync.dma_start(out=b2s[:], in_=b2col[:])
            nc.sync.dma_start(out=b2row[:], in_=b2r[:])
            nc.vector.memset(onesrow[:], 1.0)

            # ---- P1/P2: parallel precompute GEMMs ----
            n_nc = tok // 512 if tok >= 512 else 1
            ncols = tok // n_nc
            with tc.tile_pool(name="p1pool", bufs=1) as pp, \
                 tc.tile_pool(name="p1work", bufs=3) as pw, \
                 tc.tile_pool(name="p1ps", bufs=4, space="PSUM") as pps:
                mcomp = pp.tile([128, 2 * MT * 128], F16)
                mcompb = pp.tile([1, MT * 128], F16)
                w1p = pp.tile([D_PHYS + 1, 4 * 128], F16)
                mainsb = pp.tile([128, 2 * tok], F16)
                mainsb1 = pp.tile([1, tok], F16)
                physsb = pp.tile([D_PHYS + 1, tok], F16)
                nc.sync.dma_start(out=mcomp[:], in_=McompT[:])
                nc.sync.dma_start(out=mcompb[:], in_=McompB[:])
                nc.sync.dma_start(out=w1p[:], in_=W1pT[:])
                nc.sync.dma_start(out=mainsb[:, 0:tok], in_=mainT[0:128, :])
                nc.sync.dma_start(out=mainsb[:, tok:2 * tok], in_=mainT[128:256, :])
                nc.sync.dma_start(out=mainsb1[:], in_=mainT[256:257, :])
                nc.sync.dma_start(out=physsb[:], in_=physT[:])

                for m in range(MT):
                    for n in range(n_nc):
                        ps = pps.tile([128, ncols], F32, name="ps")
                        cs = slice(n * ncols, (n + 1) * ncols)
                        nc.tensor.matmul(ps[:], mcomp[:, (0 * MT + m) * 128:(0 * MT + m + 1) * 128],
                                         mainsb[:, n * ncols:(n + 1) * ncols], start=True, stop=False)
                        nc.tensor.matmul(ps[:], mcomp[:, (1 * MT + m) * 128:(1 * MT + m + 1) * 128],
                                         mainsb[:, tok + n * ncols:tok + (n + 1) * ncols], start=False, stop=False)
                        nc.tensor.matmul(ps[:], mcompb[:, m * 128:(m + 1) * 128],
                                         mainsb1[:, cs], start=False, stop=True)
                        ev = pw.tile([128, ncols], F16, name="ev")
                        nc.scalar.activation(ev[:], ps[:], AF.Copy)
                        nc.sync.dma_start(
                            out=pre0_d[:, ds(n * (ncols // BL), ncols // BL), ts(m, BL)],
                            in_=ev[:].rearrange("p (t b) -> p t b", b=BL))
                for m in range(4):
                    for n in range(n_nc):
                        ps = pps.tile([128, ncols], F32, name="ps")
                        nc.tensor.matmul(ps[:], w1p[:, m * 128:(m + 1) * 128],
                                         physsb[:, n * ncols:(n + 1) * ncols], start=True, stop=True)
                        ev = pw.tile([128, ncols], F16, name="ev")
                        nc.scalar.activation(ev[:], ps[:], AF.Copy)
                        nc.sync.dma_start(
                            out=preh_d[:, ds(n * (ncols // BL), ncols // BL), ts(m, BL)],
                            in_=ev[:].rearrange("p (t b) -> p t b", b=BL))

            # ---- P4: the sequential scan ----
            with tc.tile_pool(name="state", bufs=1) as st, \
                 tc.tile_pool(name="sw", bufs=3) as sw, \
                 tc.tile_pool(name="sps", bufs=1, space="PSUM") as sps:
                hT0 = st.tile([128, 4 * BL], F16)
                hT1 = st.tile([128, 4 * BL], F16)
                c0 = st.tile([128, 4 * BL], F32)
                c1 = st.tile([128, 4 * BL], F32)
                onehot = st.tile([C, BL], F16)
                nc.vector.memset(hT0[:], 0.0)
                nc.vector.memset(hT1[:], 0.0)
                nc.vector.memset(c0[:], 0.0)
                nc.vector.memset(c1[:], 0.0)
                nc.vector.memset(onehot[:], 0.0)
                nc.vector.memset(onehot[0:1, :], 1.0)

                QB = 4 * BL  # quarter width in gate-bank columns (32)

                unroll = 1
                assert (reps * t_steps) % unroll == 0

                def step(i):
                    pre0_t = sw.tile([128, MT * BL], F16, name="pre0_t")
                    nc.sync.dma_start(out=pre0_t[:], in_=pre0_d[:, ds(i, 1), :].opt())
                    preh_t = sw.tile([128, 4 * BL], F16, name="preh_t")
                    nc.sync.dma_start(out=preh_t[:], in_=preh_d[:, ds(i, 1), :].opt())

                    # gates layer 0 (Whh0 first: no dependence on this
                    # step's DMA loads, so pre0_t/preh_t prefetch overlaps)
                    g0 = sps.tile([128, MT * BL], F32, name="g0", bufs=2)
                    for m in range(MT):
                        for k in range(4):
                            nc.tensor.matmul(g0[:, ts(m, BL)],
                                             whh0[:, ((k * MT) + m) * 128:((k * MT) + m + 1) * 128],
                                             hT0[:, ts(k, BL)], start=(m == 0 and k == 0), stop=False)
                    nc.tensor.matmul(g0[:], idt[:], pre0_t[:], start=False, stop=False)
                    for m in range(MT):
                        nc.tensor.matmul(g0[:, ts(m, BL)], e0[:, m * 128:(m + 1) * 128],
                                         onehot[:], start=False, stop=(m == MT - 1))

                    # cell 0 elementwise:  quarters i|f|g|o at col blocks
                    sif = sw.tile([128, MT * BL], F32, name="sif")
                    nc.scalar.activation(sif[:], g0[:], AF.Sigmoid)
                    tg0 = sw.tile([128, QB], F32, name="tg0")
                    nc.scalar.activation(tg0[:], g0[:, 2 * QB:3 * QB], AF.Tanh)
                    t1 = sw.tile([128, QB], F32, name="t1")
                    nc.vector.tensor_mul(t1[:], sif[:, 0:QB], tg0[:])
                    nc.vector.tensor_mul(c0[:], c0[:], sif[:, QB:2 * QB])
                    nc.vector.tensor_add(c0[:], c0[:], t1[:])
                    tc0 = sw.tile([128, QB], F32, name="tc0")
                    nc.scalar.activation(tc0[:], c0[:], AF.Tanh)
                    nc.vector.tensor_mul(hT0[:], sif[:, 3 * QB:4 * QB], tc0[:])

                    # gates layer 1
                    g1 = sps.tile([128, MT * BL], F32, name="g1", bufs=2)
                    nc.tensor.matmul(g1[:], idt[:], bias1[:], start=True, stop=False)
                    for m in range(MT):
                        for k in range(4):
                            nc.tensor.matmul(g1[:, ts(m, BL)],
                                             whh1[:, ((k * MT) + m) * 128:((k * MT) + m + 1) * 128],
                                             hT1[:, ts(k, BL)], start=False, stop=False)
                    for m in range(MT):
                        for k in range(4):
                            nc.tensor.matmul(g1[:, ts(m, BL)],
                                             wih1[:, ((k * MT) + m) * 128:((k * MT) + m + 1) * 128],
                                             hT0[:, ts(k, BL)], start=False,
                                             stop=(m == MT - 1 and k == 3))

                    sif1 = sw.tile([128, MT * BL], F32, name="sif1")
                    nc.scalar.activation(sif1[:], g1[:], AF.Sigmoid)
                    tg1 = sw.tile([128, QB], F32, name="tg1")
                    nc.scalar.activation(tg1[:], g1[:, 2 * QB:3 * QB], AF.Tanh)
                    t2 = sw.tile([128, QB], F32, name="t2")
                    nc.vector.tensor_mul(t2[:], sif1[:, 0:QB], tg1[:])
                    nc.vector.tensor_mul(c1[:], c1[:], sif1[:, QB:2 * QB])
                    nc.vector.tensor_add(c1[:], c1[:], t2[:])
                    tc1 = sw.tile([128, QB], F32, name="tc1")
                    nc.scalar.activation(tc1[:], c1[:], AF.Tanh)
                    nc.vector.tensor_mul(hT1[:], sif1[:, 3 * QB:4 * QB], tc1[:])

                    # head:  relu(W1h @ h1 + preh) -> logits
                    hps = sps.tile([128, 4 * BL], F32, name="hps")
                    nc.tensor.matmul(hps[:], idt[:], preh_t[:], start=True, stop=False)
                    for m in range(4):
                        for k in range(4):
                            nc.tensor.matmul(hps[:, ts(m, BL)],
                                             w1h[:, ((k * 4) + m) * 128:((k * 4) + m + 1) * 128],
                                             hT1[:, ts(k, BL)], start=False,
                                             stop=(m == 3 and k == 3))
                    relu = sw.tile([128, 4 * BL], F16, name="relu")
                    nc.scalar.activation(relu[:], hps[:], AF.Relu)

                    lg = sps.tile([C, BL], F32, name="lg")
                    for k in range(4):
                        nc.tensor.matmul(lg[:], w2[:, k * C:(k + 1) * C],
                                         relu[:, ts(k, BL)], start=(k == 0), stop=(k == 3))
                    logits = sw.tile([C, BL], F32, name="logits")
                    nc.vector.tensor_scalar_add(logits[:], lg[:], b2s[:])
                    nc.sync.dma_start(out=out_d[ds(i, 1), :, :].opt(), in_=logits[:])

                    # argmax -> onehot: batch-major logits via tiny matmuls
                    # (relu chunks as stationary), free-dim max, is_ge, then
                    # one PE transpose back to feature-major.
                    lgb = sps.tile([BL, C], F32, name="lgb")
                    for k in range(4):
                        nc.tensor.matmul(lgb[:], relu[:, ts(k, BL)],
                                         w2[:, k * C:(k + 1) * C],
                                         start=(k == 0), stop=False)
                    nc.tensor.matmul(lgb[:], onesrow[:], b2row[:],
                                     start=False, stop=True)
                    mx1 = sw.tile([BL, 1], F32, name="mx1")
                    nc.vector.tensor_reduce(mx1[:], lgb[:], mybir.AxisListType.X, mybir.AluOpType.max)
                    ohb = sw.tile([BL, C], F16, name="ohb")
                    nc.vector.tensor_scalar(ohb[:], lgb[:], mx1[:], None,
                                            mybir.AluOpType.is_ge)
                    ohp = sps.tile([C, BL], F16, name="ohp")
                    nc.tensor.transpose(ohp[:], ohb[:], idt[0:BL, 0:BL])
                    nc.vector.tensor_copy(onehot[:], ohp[:])

                with tc.For_i(0, (reps * t_steps) // unroll, staggered_reset=True) as i2:
                    for u in range(unroll):
                        tt = i2 * unroll + u
                        if reps != 1:
                            tt = nc.snap(tt % t_steps)
                        step(tt)

    nc.finalize()
    return nc


def _prep_maps(inputs: dict, t_steps: int):
    f32 = np.float32
    main_feats = np.asarray(inputs["main_feats"], f32)
    phys_feats = np.asarray(inputs["phys_feats"], f32)
    Wp = np.asarray(inputs["Wp"], f32)
    emb = np.asarray(inputs["emb"], f32)
    Wih0 = np.asarray(inputs["Wih0"], f32)
    Whh0 = np.asarray(inputs["Whh0"], f32)
    bih0 = np.asarray(inputs["bih0"], f32)
    bhh0 = np.asarray(inputs["bhh0"], f32)
    Wih1 = np.asarray(inputs["Wih1"], f32)
    Whh1 = np.asarray(inputs["Whh1"], f32)
    bih1 = np.asarray(inputs["bih1"], f32)
    bhh1 = np.asarray(inputs["bhh1"], f32)
    W1 = np.asarray(inputs["W1"], f32)
    b1 = np.asarray(inputs["b1"], f32)
    W2 = np.asarray(inputs["W2"], f32)
    b2 = np.asarray(inputs["b2"], f32)

    Wih0_m, Wih0_e = Wih0[:, :H], Wih0[:, H:]
    Mcomp = Wih0_m @ Wp                      # [G, D_IN]
    b0f = bih0 + bhh0                        # folded into P1 via aug row
    Mcomp_aug_T = np.concatenate([Mcomp, b0f[:, None]], axis=1).T  # [D_IN+1, G]
    McompT_v = _tile_stationary(Mcomp_aug_T[:256, :])
    McompB_v = Mcomp_aug_T[256:257, :].astype(np.float16)

    E0 = Wih0_e @ emb.T                      # [G, C]
    E0T_v = np.ascontiguousarray(E0.T).astype(np.float16)  # [C, G]

    b1f = bih1 + bhh1                        # [G]
    BIAS1_v = np.repeat(b1f.reshape(MT, 128).T[:, :, None], BL, axis=2).reshape(
        128, MT * BL).astype(np.float16)

    W1_h, W1_p = W1[:, :H], W1[:, H:]
    W1p_aug_T = np.concatenate([W1_p, b1[:, None]], axis=1).T   # [33, 512]
    W1pT_v = np.ascontiguousarray(W1p_aug_T).astype(np.float16)

    Whh0T_v = _tile_stationary(Whh0.T)
    Wih1T_v = _tile_stationary(Wih1.T)
    Whh1T_v = _tile_stationary(Whh1.T)
    W1hT_v = _tile_stationary(W1_h.T)
    W2T_v = (W2.T.reshape(4, 128, C).transpose(1, 0, 2).reshape(128, 4 * C)
             ).astype(np.float16)
    IDT_v = np.eye(128, dtype=np.float16)
    b2col_v = b2[:, None].astype(f32)

    b2r_v = b2[None, :].astype(np.float16)
    shared = dict(b2r=b2r_v, McompT=McompT_v, McompB=McompB_v, W1pT=W1pT_v, Whh0T=Whh0T_v,
                  Wih1T=Wih1T_v, Whh1T=Whh1T_v, W1hT=W1hT_v, W2T=W2T_v,
                  E0T=E0T_v, BIAS1=BIAS1_v, IDT=IDT_v, b2col=b2col_v)

    tok = t_steps * BL
    in_maps = []
    for j in range(NCORES):
        bsl = slice(j * BL, (j + 1) * BL)
        mj = main_feats[bsl, :t_steps]           # [BL, t, D_IN]
        mT = mj.transpose(2, 1, 0).reshape(D_IN, tok)          # col = t*BL + b
        mT = np.concatenate([mT, np.ones((1, tok), f32)], axis=0).astype(np.float16)
        pj = phys_feats[bsl, :t_steps]
        pT = pj.transpose(2, 1, 0).reshape(D_PHYS, tok)
        pT = np.concatenate([pT, np.ones((1, tok), f32)], axis=0).astype(np.float16)
        in_maps.append(dict(shared, mainT=np.ascontiguousarray(mT),
                            physT=np.ascontiguousarray(pT)))
    return in_maps


_BUILD_CACHE = {}


def get_built(t_steps: int, reps: int = 1):
    key = (t_steps, reps)
    if key not in _BUILD_CACHE:
        _BUILD_CACHE[key] = build(t_steps, reps)
    return _BUILD_CACHE[key]


def kernel(**inputs):
    t_steps = int(os.environ.get("ARLSTM_T", T))
    trace = bool(int(os.environ.get("ARLSTM_TRACE", "0")))
    nc = get_built(t_steps)
    in_maps = _prep_maps(inputs, t_steps)
    res = run_bass_kernel_spmd(nc, in_maps, core_ids=list(range(NCORES)),
                               trace=trace)
    kernel.last_result = res
    logits_full = np.empty((B, t_steps, C), np.float32)
    for j in range(NCORES):
        lg = res.results[j]["out_logits"]        # [t, C, BL]
        logits_full[j * BL:(j + 1) * BL] = lg.transpose(2, 0, 1)
    full = np.zeros((B, T, C), np.float32)
    full[:, :t_steps] = logits_full
    if t_steps != T:
        return full[:, :t_steps]
    attn = np.zeros((B, T), np.float32)
    return full, np.zeros_like(full), attn


# revision 14
# speedup vs baseline: 1.0250x; 1.0250x over previous
"""Trainium2 Bass kernel for the autoregressive 2-layer LSTM (nn_ArLSTM).

Strategy (phase A): data-parallel over batch. B=64 is sharded 8 ways (8
sequences per core); each core runs the full T=512 sequential scan locally
with no cross-core communication.

Algebraic restructuring vs the reference:
  - x_main @ Wih0_m is composed:  Wih0[:, :H] @ Wp  is one [4H, D_IN] matrix,
    so the per-step input term  pre0[t] = (Wih0_m@Wp) @ main_t + bih0 + bhh0
    is a parallel GEMM over all (b, t), done on-device before the scan.
  - emb[prev] enters only through Wih0[:, H:] @ emb[prev].  E0 = Wih0_e@emb.T
    is a [4H, 11] matrix; the per-step term is E0 @ onehot(prev), a K=11
    matmul.  onehot is built from the logits with a max-compare (no gather).
  - The head's phys term  W1[:, H:] @ ph_t + b1  is precomputed per (b, t).
  - All matmuls run in fp16 (1 cycle/row on PE) with fp32 PSUM accumulation;
    cell state c and all element-wise math stay fp32.  Measured end-to-end
    rel_err vs the fp32 reference ~6.5e-3 (argmax feedback is contractive).

Gate-bank layout: gates [4H, B_local] live in one PSUM bank [128, 16*BL]
where column-block m holds gate rows 128m..128m+127.  With PyTorch gate
order (i, f, g, o) the quarters are column ranges, so the whole LSTM cell
is element-wise ops on [128, 4*BL] slices.
"""

import os
import numpy as np

import concourse.bass as bass
import concourse.tile as tile
from concourse import bacc, mybir
from concourse.bass import ds, ts
from concourse.bass_utils import run_bass_kernel_spmd

F16 = mybir.dt.float16
F32 = mybir.dt.float32

B, T, D_IN, D_PHYS, H, C = 64, 512, 256, 32, 512, 11
NCORES = 8
BL = B // NCORES          # 8 sequences per core
G = 4 * H                 # 2048 gate rows
MT = G // 128             # 16 gate m-tiles
TOK = None                # set per build (t_steps * BL)


def _tile_stationary(wt: np.ndarray) -> np.ndarray:
    """[K, M] -> [128, (K/128)*(M/128)*128] fp16 stationary-tile layout.

    Free index = ((k_chunk*MT_loc + m_tile)*128 + col)."""
    K, M = wt.shape
    kc, mt = K // 128, M // 128
    return (
        wt.reshape(kc, 128, mt, 128).transpose(1, 0, 2, 3).reshape(128, kc * mt * 128)
    ).astype(np.float16)


def build(t_steps: int, reps: int = 1):
    tok = t_steps * BL
    nc = bacc.Bacc(None, target_bir_lowering=False, debug=False)

    # ---- DRAM parameters (per-core inputs) ----
    mainT = nc.declare_dram_parameter("mainT", [D_IN + 1, tok], F16, isOutput=False)    # aug ones row
    physT = nc.declare_dram_parameter("physT", [D_PHYS + 1, tok], F16, isOutput=False)  # aug ones row
    McompT = nc.declare_dram_parameter("McompT", [128, 2 * MT * 128], F16, isOutput=False)
    McompB = nc.declare_dram_parameter("McompB", [1, MT * 128], F16, isOutput=False)    # bias row
    W1pT = nc.declare_dram_parameter("W1pT", [D_PHYS + 1, 4 * 128], F16, isOutput=False)
    Whh0T = nc.declare_dram_parameter("Whh0T", [128, 4 * MT * 128], F16, isOutput=False)
    Wih1T = nc.declare_dram_parameter("Wih1T", [128, 4 * MT * 128], F16, isOutput=False)
    Whh1T = nc.declare_dram_parameter("Whh1T", [128, 4 * MT * 128], F16, isOutput=False)
    W1hT = nc.declare_dram_parameter("W1hT", [128, 4 * 4 * 128], F16, isOutput=False)
    W2T = nc.declare_dram_parameter("W2T", [128, 4 * C], F16, isOutput=False)
    E0T = nc.declare_dram_parameter("E0T", [C, MT * 128], F16, isOutput=False)
    BIAS1 = nc.declare_dram_parameter("BIAS1", [128, MT * BL], F16, isOutput=False)
    IDT = nc.declare_dram_parameter("IDT", [128, 128], F16, isOutput=False)
    b2col = nc.declare_dram_parameter("b2col", [C, 1], F32, isOutput=False)
    b2r = nc.declare_dram_parameter("b2r", [1, C], F16, isOutput=False)

    pre0_d = nc.dram_tensor("pre0_d", [128, t_steps, MT * BL], F16)
    preh_d = nc.dram_tensor("preh_d", [128, t_steps, 4 * BL], F16)
    out_d = nc.declare_dram_parameter("out_logits", [t_steps, C, BL], F32, isOutput=True)

    AF = mybir.ActivationFunctionType

    with tile.TileContext(nc) as tc:
        with tc.tile_pool(name="wpool", bufs=1) as wp:
            # persistent weights in SBUF
            whh0 = wp.tile([128, 4 * MT * 128], F16)
            wih1 = wp.tile([128, 4 * MT * 128], F16)
            whh1 = wp.tile([128, 4 * MT * 128], F16)
            w1h = wp.tile([128, 4 * 4 * 128], F16)
            w2 = wp.tile([128, 4 * C], F16)
            e0 = wp.tile([C, MT * 128], F16)
            bias1 = wp.tile([128, MT * BL], F16)
            idt = wp.tile([128, 128], F16)
            b2s = wp.tile([C, 1], F32)
            b2row = wp.tile([1, C], F16)
            onesrow = wp.tile([1, BL], F16)
            nc.sync.dma_start(out=whh0[:], in_=Whh0T[:])
            nc.sync.dma_start(out=wih1[:], in_=Wih1T[:])
            nc.sync.dma_start(out=whh1[:], in_=Whh1T[:])
            nc.sync.dma_start(out=w1h[:], in_=W1hT[:])
            nc.sync.dma_start(out=w2[:], in_=W2T[:])
            nc.sync.dma_start(out=e0[:], in_=E0T[:])
            nc.sync.dma_start(out=bias1[:], in_=BIAS1[:])
            nc.sync.dma_start(out=idt[:], in_=IDT[:])
            nc.sync.dma_start(out=b2s[:], in_=b2col[:])
            nc.sync.dma_start(out=b2row[:], in_=b2r[:])
            nc.vector.memset(onesrow[:], 1.0)

            # ---- P1/P2: parallel precompute GEMMs ----
            n_nc = tok // 512 if tok >= 512 else 1
            ncols = tok // n_nc
            with tc.tile_pool(name="p1pool", bufs=1) as pp, \
                 tc.tile_pool(name="p1work", bufs=3) as pw, \
                 tc.tile_pool(name="p1ps", bufs=4, space="PSUM") as pps:
                mcomp = pp.tile([128, 2 * MT * 128], F16)
                mcompb = pp.tile([1, MT * 128], F16)
                w1p = pp.tile([D_PHYS + 1, 4 * 128], F16)
                mainsb = pp.tile([128, 2 * tok], F16)
                mainsb1 = pp.tile([1, tok], F16)
                physsb = pp.tile([D_PHYS + 1, tok], F16)
                nc.sync.dma_start(out=mcomp[:], in_=McompT[:])
                nc.sync.dma_start(out=mcompb[:], in_=McompB[:])
                nc.sync.dma_start(out=w1p[:], in_=W1pT[:])
                nc.sync.dma_start(out=mainsb[:, 0:tok], in_=mainT[0:128, :])
                nc.sync.dma_start(out=mainsb[:, tok:2 * tok], in_=mainT[128:256, :])
                nc.sync.dma_start(out=mainsb1[:], in_=mainT[256:257, :])
                nc.sync.dma_start(out=physsb[:], in_=physT[:])

                for m in range(MT):
                    for n in range(n_nc):
                        ps = pps.tile([128, ncols], F32, name="ps")
                        cs = slice(n * ncols, (n + 1) * ncols)
                        nc.tensor.matmul(ps[:], mcomp[:, (0 * MT + m) * 128:(0 * MT + m + 1) * 128],
                                         mainsb[:, n * ncols:(n + 1) * ncols], start=True, stop=False)
                        nc.tensor.matmul(ps[:], mcomp[:, (1 * MT + m) * 128:(1 * MT + m + 1) * 128],
                                         mainsb[:, tok + n * ncols:tok + (n + 1) * ncols], start=False, stop=False)
                        nc.tensor.matmul(ps[:], mcompb[:, m * 128:(m + 1) * 128],
                                         mainsb1[:, cs], start=False, stop=True)
                        ev = pw.tile([128, ncols], F16, name="ev")
                        nc.scalar.activation(ev[:], ps[:], AF.Copy)
                        nc.sync.dma_start(
                            out=pre0_d[:, ds(n * (ncols // BL), ncols // BL), ts(m, BL)],
                            in_=ev[:].rearrange("p (t b) -> p t b", b=BL))
                for m in range(4):
                    for n in range(n_nc):
                        ps = pps.tile([128, ncols], F32, name="ps")
                        nc.tensor.matmul(ps[:], w1p[:, m * 128:(m + 1) * 128],
                                         physsb[:, n * ncols:(n + 1) * ncols], start=True, stop=True)
                        ev = pw.tile([128, ncols], F16, name="ev")
                        nc.scalar.activation(ev[:], ps[:], AF.Copy)
                        nc.sync.dma_start(
                            out=preh_d[:, ds(n * (ncols // BL), ncols // BL), ts(m, BL)],
                            in_=ev[:].rearrange("p (t b) -> p t b", b=BL))

            # ---- P4: the sequential scan ----
            with tc.tile_pool(name="state", bufs=1) as st, \
                 tc.tile_pool(name="sw", bufs=3) as sw, \
                 tc.tile_pool(name="sps", bufs=1, space="PSUM") as sps:
                hT0 = st.tile([128, 4 * BL], F16)
                hT1 = st.tile([128, 4 * BL], F16)
                c0 = st.tile([128, 4 * BL], F32)
                c1 = st.tile([128, 4 * BL], F32)
                onehot = st.tile([C, BL], F16)
                nc.vector.memset(hT0[:], 0.0)
                nc.vector.memset(hT1[:], 0.0)
                nc.vector.memset(c0[:], 0.0)
                nc.vector.memset(c1[:], 0.0)
                nc.vector.memset(onehot[:], 0.0)
                nc.vector.memset(onehot[0:1, :], 1.0)

                QB = 4 * BL  # quarter width in gate-bank columns (32)

                unroll = 1
                assert (reps * t_steps) % unroll == 0

                def step(i):
                    pre0_t = sw.tile([128, MT * BL], F16, name="pre0_t")
                    nc.sync.dma_start(out=pre0_t[:], in_=pre0_d[:, ds(i, 1), :].opt())
                    preh_t = sw.tile([128, 4 * BL], F16, name="preh_t")
                    nc.sync.dma_start(out=preh_t[:], in_=preh_d[:, ds(i, 1), :].opt())

                    # gates layer 0 (Whh0 first: no dependence on this
                    # step's DMA loads, so pre0_t/preh_t prefetch overlaps)
                    g0 = sps.tile([128, MT * BL], F32, name="g0")
                    for m in range(MT):
                        for k in range(4):
                            nc.tensor.matmul(g0[:, ts(m, BL)],
                                             whh0[:, ((k * MT) + m) * 128:((k * MT) + m + 1) * 128],
                                             hT0[:, ts(k, BL)], start=(m == 0 and k == 0), stop=False)
                    nc.tensor.matmul(g0[:], idt[:], pre0_t[:], start=False, stop=False)
                    for m in range(MT):
                        nc.tensor.matmul(g0[:, ts(m, BL)], e0[:, m * 128:(m + 1) * 128],
                                         onehot[:], start=False, stop=(m == MT - 1))

                    # cell 0 elementwise:  quarters i|f|g|o at col blocks
                    sif = sw.tile([128, MT * BL], F32, name="sif")
                    nc.scalar.activation(sif[:], g0[:], AF.Sigmoid)
                    tg0 = sw.tile([128, QB], F32, name="tg0")
                    nc.scalar.activation(tg0[:], g0[:, 2 * QB:3 * QB], AF.Tanh)
                    t1 = sw.tile([128, QB], F32, name="t1")
                    nc.vector.tensor_mul(t1[:], sif[:, 0:QB], tg0[:])
                    nc.vector.tensor_mul(c0[:], c0[:], sif[:, QB:2 * QB])
                    nc.vector.tensor_add(c0[:], c0[:], t1[:])
                    tc0 = sw.tile([128, QB], F32, name="tc0")
                    nc.scalar.activation(tc0[:], c0[:], AF.Tanh)
                    nc.vector.tensor_mul(hT0[:], sif[:, 3 * QB:4 * QB], tc0[:])

                    # gates layer 1
                    g1 = sps.tile([128, MT * BL], F32, name="g1")
                    nc.tensor.matmul(g1[:], idt[:], bias1[:], start=True, stop=False)
                    for m in range(MT):
                        for k in range(4):
                            nc.tensor.matmul(g1[:, ts(m, BL)],
                                             whh1[:, ((k * MT) + m) * 128:((k * MT) + m + 1) * 128],
                                             hT1[:, ts(k, BL)], start=False, stop=False)
                    for m in range(MT):
                        for k in range(4):
                            nc.tensor.matmul(g1[:, ts(m, BL)],
                                             wih1[:, ((k * MT) + m) * 128:((k * MT) + m + 1) * 128],
                                             hT0[:, ts(k, BL)], start=False,
                                             stop=(m == MT - 1 and k == 3))

                    sif1 = sw.tile([128, MT * BL], F32, name="sif1")
                    nc.scalar.activation(sif1[:], g1[:], AF.Sigmoid)
                    tg1 = sw.tile([128, QB], F32, name="tg1")
                    nc.scalar.activation(tg1[:], g1[:, 2 * QB:3 * QB], AF.Tanh)
                    t2 = sw.tile([128, QB], F32, name="t2")
                    nc.vector.tensor_mul(t2[:], sif1[:, 0:QB], tg1[:])
                    nc.vector.tensor_mul(c1[:], c1[:], sif1[:, QB:2 * QB])
                    nc.vector.tensor_add(c1[:], c1[:], t2[:])
                    tc1 = sw.tile([128, QB], F32, name="tc1")
                    nc.scalar.activation(tc1[:], c1[:], AF.Tanh)
                    nc.vector.tensor_mul(hT1[:], sif1[:, 3 * QB:4 * QB], tc1[:])

                    # head:  relu(W1h @ h1 + preh) -> logits
                    hps = sps.tile([128, 4 * BL], F32, name="hps")
                    nc.tensor.matmul(hps[:], idt[:], preh_t[:], start=True, stop=False)
                    for m in range(4):
                        for k in range(4):
                            nc.tensor.matmul(hps[:, ts(m, BL)],
                                             w1h[:, ((k * 4) + m) * 128:((k * 4) + m + 1) * 128],
                                             hT1[:, ts(k, BL)], start=False,
                                             stop=(m == 3 and k == 3))
                    relu = sw.tile([128, 4 * BL], F16, name="relu")
                    nc.scalar.activation(relu[:], hps[:], AF.Relu)

                    lg = sps.tile([C, BL], F32, name="lg")
                    for k in range(4):
                        nc.tensor.matmul(lg[:], w2[:, k * C:(k + 1) * C],
                                         relu[:, ts(k, BL)], start=(k == 0), stop=(k == 3))
                    logits = sw.tile([C, BL], F32, name="logits")
                    nc.vector.tensor_scalar_add(logits[:], lg[:], b2s[:])
                    nc.sync.dma_start(out=out_d[ds(i, 1), :, :].opt(), in_=logits[:])

                    # argmax -> onehot: batch-major logits via tiny matmuls
                    # (relu chunks as stationary), free-dim max, is_ge, then
                    # one PE transpose back to feature-major.
                    lgb = sps.tile([BL, C], F32, name="lgb")
                    for k in range(4):
                        nc.tensor.matmul(lgb[:], relu[:, ts(k, BL)],
                                         w2[:, k * C:(k + 1) * C],
                                         start=(k == 0), stop=False)
                    nc.tensor.matmul(lgb[:], onesrow[:], b2row[:],
                                     start=False, stop=True)
                    mx1 = sw.tile([BL, 1], F32, name="mx1")
                    nc.vector.tensor_reduce(mx1[:], lgb[:], mybir.AxisListType.X, mybir.AluOpType.max)
                    ohb = sw.tile([BL, C], F16, name="ohb")
                    nc.vector.tensor_scalar(ohb[:], lgb[:], mx1[:], None,
                                            mybir.AluOpType.is_ge)
                    ohp = sps.tile([C, BL], F16, name="ohp")
                    nc.tensor.transpose(ohp[:], ohb[:], idt[0:BL, 0:BL])
                    nc.vector.tensor_copy(onehot[:], ohp[:])

                with tc.For_i(0, (reps * t_steps) // unroll, staggered_reset=True) as i2:
                    for u in range(unroll):
                        tt = i2 * unroll + u
                        if reps != 1:
                            tt = nc.snap(tt % t_steps)
                        step(tt)

    nc.finalize()
    return nc


def _prep_maps(inputs: dict, t_steps: int):
    f32 = np.float32
    main_feats = np.asarray(inputs["main_feats"], f32)
    phys_feats = np.asarray(inputs["phys_feats"], f32)
    Wp = np.asarray(inputs["Wp"], f32)
    emb = np.asarray(inputs["emb"], f32)
    Wih0 = np.asarray(inputs["Wih0"], f32)
    Whh0 = np.asarray(inputs["Whh0"], f32)
    bih0 = np.asarray(inputs["bih0"], f32)
    bhh0 = np.asarray(inputs["bhh0"], f32)
    Wih1 = np.asarray(inputs["Wih1"], f32)
    Whh1 = np.asarray(inputs["Whh1"], f32)
    bih1 = np.asarray(inputs["bih1"], f32)
    bhh1 = np.asarray(inputs["bhh1"], f32)
    W1 = np.asarray(inputs["W1"], f32)
    b1 = np.asarray(inputs["b1"], f32)
    W2 = np.asarray(inputs["W2"], f32)
    b2 = np.asarray(inputs["b2"], f32)

    Wih0_m, Wih0_e = Wih0[:, :H], Wih0[:, H:]
    Mcomp = Wih0_m @ Wp                      # [G, D_IN]
    b0f = bih0 + bhh0                        # folded into P1 via aug row
    Mcomp_aug_T = np.concatenate([Mcomp, b0f[:, None]], axis=1).T  # [D_IN+1, G]
    McompT_v = _tile_stationary(Mcomp_aug_T[:256, :])
    McompB_v = Mcomp_aug_T[256:257, :].astype(np.float16)

    E0 = Wih0_e @ emb.T                      # [G, C]
    E0T_v = np.ascontiguousarray(E0.T).astype(np.float16)  # [C, G]

    b1f = bih1 + bhh1                        # [G]
    BIAS1_v = np.repeat(b1f.reshape(MT, 128).T[:, :, None], BL, axis=2).reshape(
        128, MT * BL).astype(np.float16)

    W1_h, W1_p = W1[:, :H], W1[:, H:]
    W1p_aug_T = np.concatenate([W1_p, b1[:, None]], axis=1).T   # [33, 512]
    W1pT_v = np.ascontiguousarray(W1p_aug_T).astype(np.float16)

    Whh0T_v = _tile_stationary(Whh0.T)
    Wih1T_v = _tile_stationary(Wih1.T)
    Whh1T_v = _tile_stationary(Whh1.T)
    W1hT_v = _tile_stationary(W1_h.T)
    W2T_v = (W2.T.reshape(4, 128, C).transpose(1, 0, 2).reshape(128, 4 * C)
             ).astype(np.float16)
    IDT_v = np.eye(128, dtype=np.float16)
    b2col_v = b2[:, None].astype(f32)

    b2r_v = b2[None, :].astype(np.float16)
    shared = dict(b2r=b2r_v, McompT=McompT_v, McompB=McompB_v, W1pT=W1pT_v, Whh0T=Whh0T_v,
                  Wih1T=Wih1T_v, Whh1T=Whh1T_v, W1hT=W1hT_v, W2T=W2T_v,
                  E0T=E0T_v, BIAS1=BIAS1_v, IDT=IDT_v, b2col=b2col_v)

    tok = t_steps * BL
    in_maps = []
    for j in range(NCORES):
        bsl = slice(j * BL, (j + 1) * BL)
        mj = main_feats[bsl, :t_steps]           # [BL, t, D_IN]
        mT = mj.transpose(2, 1, 0).reshape(D_IN, tok)          # col = t*BL + b
        mT = np.concatenate([mT, np.ones((1, tok), f32)], axis=0).astype(np.float16)
        pj = phys_feats[bsl, :t_steps]
        pT = pj.transpose(2, 1, 0).reshape(D_PHYS, tok)
        pT = np.concatenate([pT, np.ones((1, tok), f32)], axis=0).astype(np.float16)
        in_maps.append(dict(shared, mainT=np.ascontiguousarray(mT),
                            physT=np.ascontiguousarray(pT)))
    return in_maps


_BUILD_CACHE = {}


def get_built(t_steps: int, reps: int = 1):
    key = (t_steps, reps)
    if key not in _BUILD_CACHE:
        _BUILD_CACHE[key] = build(t_steps, reps)
    return _BUILD_CACHE[key]


def kernel(**inputs):
    t_steps = int(os.environ.get("ARLSTM_T", T))
    trace = bool(int(os.environ.get("ARLSTM_TRACE", "0")))
    nc = get_built(t_steps)
    in_maps = _prep_maps(inputs, t_steps)
    res = run_bass_kernel_spmd(nc, in_maps, core_ids=list(range(NCORES)),
                               trace=trace)
    kernel.last_result = res
    logits_full = np.empty((B, t_steps, C), np.float32)
    for j in range(NCORES):
        lg = res.results[j]["out_logits"]        # [t, C, BL]
        logits_full[j * BL:(j + 1) * BL] = lg.transpose(2, 0, 1)
    full = np.zeros((B, T, C), np.float32)
    full[:, :t_steps] = logits_full
    if t_steps != T:
        return full[:, :t_steps]
    attn = np.zeros((B, T), np.float32)
    return full, np.zeros_like(full), attn


# revision 15
# speedup vs baseline: 1.0770x; 1.0507x over previous
"""Trainium2 Bass kernel for the autoregressive 2-layer LSTM (nn_ArLSTM).

Strategy (phase A): data-parallel over batch. B=64 is sharded 8 ways (8
sequences per core); each core runs the full T=512 sequential scan locally
with no cross-core communication.

Algebraic restructuring vs the reference:
  - x_main @ Wih0_m is composed:  Wih0[:, :H] @ Wp  is one [4H, D_IN] matrix,
    so the per-step input term  pre0[t] = (Wih0_m@Wp) @ main_t + bih0 + bhh0
    is a parallel GEMM over all (b, t), done on-device before the scan.
  - emb[prev] enters only through Wih0[:, H:] @ emb[prev].  E0 = Wih0_e@emb.T
    is a [4H, 11] matrix; the per-step term is E0 @ onehot(prev), a K=11
    matmul.  onehot is built from the logits with a max-compare (no gather).
  - The head's phys term  W1[:, H:] @ ph_t + b1  is precomputed per (b, t).
  - All matmuls run in fp16 (1 cycle/row on PE) with fp32 PSUM accumulation;
    cell state c and all element-wise math stay fp32.  Measured end-to-end
    rel_err vs the fp32 reference ~6.5e-3 (argmax feedback is contractive).

Gate-bank layout: gates [4H, B_local] live in one PSUM bank [128, 16*BL]
where column-block m holds gate rows 128m..128m+127.  With PyTorch gate
order (i, f, g, o) the quarters are column ranges, so the whole LSTM cell
is element-wise ops on [128, 4*BL] slices.
"""

import os
import numpy as np

import concourse.bass as bass
import concourse.tile as tile
from concourse import bacc, mybir
from concourse.bass import ds, ts
from concourse.bass_utils import run_bass_kernel_spmd

F16 = mybir.dt.float16
F32 = mybir.dt.float32

B, T, D_IN, D_PHYS, H, C = 64, 512, 256, 32, 512, 11
NCORES = 8
BL = B // NCORES          # 8 sequences per core
G = 4 * H                 # 2048 gate rows
MT = G // 128             # 16 gate m-tiles
TOK = None                # set per build (t_steps * BL)


def _tile_stationary(wt: np.ndarray) -> np.ndarray:
    """[K, M] -> [128, (K/128)*(M/128)*128] fp16 stationary-tile layout.

    Free index = ((k_chunk*MT_loc + m_tile)*128 + col)."""
    K, M = wt.shape
    kc, mt = K // 128, M // 128
    return (
        wt.reshape(kc, 128, mt, 128).transpose(1, 0, 2, 3).reshape(128, kc * mt * 128)
    ).astype(np.float16)


def build(t_steps: int, reps: int = 1):
    tok = t_steps * BL
    nc = bacc.Bacc(None, target_bir_lowering=False, debug=False)

    # ---- DRAM parameters (per-core inputs) ----
    mainT = nc.declare_dram_parameter("mainT", [D_IN + 1, tok], F16, isOutput=False)    # aug ones row
    physT = nc.declare_dram_parameter("physT", [D_PHYS + 1, tok], F16, isOutput=False)  # aug ones row
    McompT = nc.declare_dram_parameter("McompT", [128, 2 * MT * 128], F16, isOutput=False)
    McompB = nc.declare_dram_parameter("McompB", [1, MT * 128], F16, isOutput=False)    # bias row
    W1pT = nc.declare_dram_parameter("W1pT", [D_PHYS + 1, 4 * 128], F16, isOutput=False)
    Whh0T = nc.declare_dram_parameter("Whh0T", [128, 4 * MT * 128], F16, isOutput=False)
    Wih1T = nc.declare_dram_parameter("Wih1T", [128, 4 * MT * 128], F16, isOutput=False)
    Whh1T = nc.declare_dram_parameter("Whh1T", [128, 4 * MT * 128], F16, isOutput=False)
    W1hT = nc.declare_dram_parameter("W1hT", [128, 4 * 4 * 128], F16, isOutput=False)
    W2T = nc.declare_dram_parameter("W2T", [128, 4 * C], F16, isOutput=False)
    E0T = nc.declare_dram_parameter("E0T", [C, MT * 128], F16, isOutput=False)
    BIAS1 = nc.declare_dram_parameter("BIAS1", [128, MT * BL], F16, isOutput=False)
    IDT = nc.declare_dram_parameter("IDT", [128, 128], F16, isOutput=False)
    b2col = nc.declare_dram_parameter("b2col", [C, 1], F32, isOutput=False)
    b2r = nc.declare_dram_parameter("b2r", [1, C], F16, isOutput=False)

    pre0_d = nc.dram_tensor("pre0_d", [128, t_steps, MT * BL], F16)
    preh_d = nc.dram_tensor("preh_d", [128, t_steps, 4 * BL], F16)
    out_d = nc.declare_dram_parameter("out_logits", [t_steps, C, BL], F32, isOutput=True)

    AF = mybir.ActivationFunctionType

    with tile.TileContext(nc) as tc:
        with tc.tile_pool(name="wpool", bufs=1) as wp:
            # persistent weights in SBUF
            whh0 = wp.tile([128, 4 * MT * 128], F16)
            wih1 = wp.tile([128, 4 * MT * 128], F16)
            whh1 = wp.tile([128, 4 * MT * 128], F16)
            w1h = wp.tile([128, 4 * 4 * 128], F16)
            w2 = wp.tile([128, 4 * C], F16)
            e0 = wp.tile([C, MT * 128], F16)
            bias1 = wp.tile([128, MT * BL], F16)
            idt = wp.tile([128, 128], F16)
            b2s = wp.tile([C, 1], F32)
            b2row = wp.tile([1, C], F16)
            onesrow = wp.tile([1, BL], F16)
            nc.sync.dma_start(out=whh0[:], in_=Whh0T[:])
            nc.sync.dma_start(out=wih1[:], in_=Wih1T[:])
            nc.sync.dma_start(out=whh1[:], in_=Whh1T[:])
            nc.sync.dma_start(out=w1h[:], in_=W1hT[:])
            nc.sync.dma_start(out=w2[:], in_=W2T[:])
            nc.sync.dma_start(out=e0[:], in_=E0T[:])
            nc.sync.dma_start(out=bias1[:], in_=BIAS1[:])
            nc.sync.dma_start(out=idt[:], in_=IDT[:])
            nc.sync.dma_start(out=b2s[:], in_=b2col[:])
            nc.sync.dma_start(out=b2row[:], in_=b2r[:])
            nc.vector.memset(onesrow[:], 1.0)

            # ---- P1/P2: parallel precompute GEMMs ----
            n_nc = tok // 512 if tok >= 512 else 1
            ncols = tok // n_nc
            with tc.tile_pool(name="p1pool", bufs=1) as pp, \
                 tc.tile_pool(name="p1work", bufs=3) as pw, \
                 tc.tile_pool(name="p1ps", bufs=4, space="PSUM") as pps:
                mcomp = pp.tile([128, 2 * MT * 128], F16)
                mcompb = pp.tile([1, MT * 128], F16)
                w1p = pp.tile([D_PHYS + 1, 4 * 128], F16)
                mainsb = pp.tile([128, 2 * tok], F16)
                mainsb1 = pp.tile([1, tok], F16)
                physsb = pp.tile([D_PHYS + 1, tok], F16)
                nc.sync.dma_start(out=mcomp[:], in_=McompT[:])
                nc.sync.dma_start(out=mcompb[:], in_=McompB[:])
                nc.sync.dma_start(out=w1p[:], in_=W1pT[:])
                nc.sync.dma_start(out=mainsb[:, 0:tok], in_=mainT[0:128, :])
                nc.sync.dma_start(out=mainsb[:, tok:2 * tok], in_=mainT[128:256, :])
                nc.sync.dma_start(out=mainsb1[:], in_=mainT[256:257, :])
                nc.sync.dma_start(out=physsb[:], in_=physT[:])

                for m in range(MT):
                    for n in range(n_nc):
                        ps = pps.tile([128, ncols], F32, name="ps")
                        cs = slice(n * ncols, (n + 1) * ncols)
                        nc.tensor.matmul(ps[:], mcomp[:, (0 * MT + m) * 128:(0 * MT + m + 1) * 128],
                                         mainsb[:, n * ncols:(n + 1) * ncols], start=True, stop=False)
                        nc.tensor.matmul(ps[:], mcomp[:, (1 * MT + m) * 128:(1 * MT + m + 1) * 128],
                                         mainsb[:, tok + n * ncols:tok + (n + 1) * ncols], start=False, stop=False)
                        nc.tensor.matmul(ps[:], mcompb[:, m * 128:(m + 1) * 128],
                                         mainsb1[:, cs], start=False, stop=True)
                        ev = pw.tile([128, ncols], F16, name="ev")
                        nc.scalar.activation(ev[:], ps[:], AF.Copy)
                        nc.sync.dma_start(
                            out=pre0_d[:, ds(n * (ncols // BL), ncols // BL), ts(m, BL)],
                            in_=ev[:].rearrange("p (t b) -> p t b", b=BL))
                for m in range(4):
                    for n in range(n_nc):
                        ps = pps.tile([128, ncols], F32, name="ps")
                        nc.tensor.matmul(ps[:], w1p[:, m * 128:(m + 1) * 128],
                                         physsb[:, n * ncols:(n + 1) * ncols], start=True, stop=True)
                        ev = pw.tile([128, ncols], F16, name="ev")
                        nc.scalar.activation(ev[:], ps[:], AF.Copy)
                        nc.sync.dma_start(
                            out=preh_d[:, ds(n * (ncols // BL), ncols // BL), ts(m, BL)],
                            in_=ev[:].rearrange("p (t b) -> p t b", b=BL))

            # ---- P4: the sequential scan ----
            with tc.tile_pool(name="state", bufs=1) as st, \
                 tc.tile_pool(name="sw", bufs=4) as sw, \
                 tc.tile_pool(name="sps", bufs=1, space="PSUM") as sps:
                hT0 = st.tile([128, 4 * BL], F16)
                hT1 = st.tile([128, 4 * BL], F16)
                c0 = st.tile([128, 4 * BL], F32)
                c1 = st.tile([128, 4 * BL], F32)
                onehot = st.tile([C, BL], F16)
                nc.vector.memset(hT0[:], 0.0)
                nc.vector.memset(hT1[:], 0.0)
                nc.vector.memset(c0[:], 0.0)
                nc.vector.memset(c1[:], 0.0)
                nc.vector.memset(onehot[:], 0.0)
                nc.vector.memset(onehot[0:1, :], 1.0)

                QB = 4 * BL  # quarter width in gate-bank columns (32)

                unroll = 1
                assert (reps * t_steps) % unroll == 0

                def step(i):
                    pre0_t = sw.tile([128, MT * BL], F16, name="pre0_t")
                    nc.sync.dma_start(out=pre0_t[:], in_=pre0_d[:, ds(i, 1), :].opt())
                    preh_t = sw.tile([128, 4 * BL], F16, name="preh_t")
                    nc.sync.dma_start(out=preh_t[:], in_=preh_d[:, ds(i, 1), :].opt())

                    # gates layer 0 (Whh0 first: no dependence on this
                    # step's DMA loads, so pre0_t/preh_t prefetch overlaps)
                    g0 = sps.tile([128, MT * BL], F32, name="g0")
                    for m in range(MT):
                        for k in range(4):
                            nc.tensor.matmul(g0[:, ts(m, BL)],
                                             whh0[:, ((k * MT) + m) * 128:((k * MT) + m + 1) * 128],
                                             hT0[:, ts(k, BL)], start=(m == 0 and k == 0), stop=False)
                    nc.tensor.matmul(g0[:], idt[:], pre0_t[:], start=False, stop=False)
                    for m in range(MT):
                        nc.tensor.matmul(g0[:, ts(m, BL)], e0[:, m * 128:(m + 1) * 128],
                                         onehot[:], start=False, stop=(m == MT - 1))

                    # cell 0 elementwise:  quarters i|f|g|o at col blocks
                    sif = sw.tile([128, MT * BL], F32, name="sif")
                    nc.scalar.activation(sif[:], g0[:], AF.Sigmoid)
                    tg0 = sw.tile([128, QB], F32, name="tg0")
                    nc.scalar.activation(tg0[:], g0[:, 2 * QB:3 * QB], AF.Tanh)
                    t1 = sw.tile([128, QB], F32, name="t1")
                    nc.vector.tensor_mul(t1[:], sif[:, 0:QB], tg0[:])
                    nc.vector.tensor_mul(c0[:], c0[:], sif[:, QB:2 * QB])
                    nc.vector.tensor_add(c0[:], c0[:], t1[:])
                    tc0 = sw.tile([128, QB], F32, name="tc0")
                    nc.scalar.activation(tc0[:], c0[:], AF.Tanh)
                    nc.vector.tensor_mul(hT0[:], sif[:, 3 * QB:4 * QB], tc0[:])

                    # gates layer 1
                    g1 = sps.tile([128, MT * BL], F32, name="g1")
                    nc.tensor.matmul(g1[:], idt[:], bias1[:], start=True, stop=False)
                    for m in range(MT):
                        for k in range(4):
                            nc.tensor.matmul(g1[:, ts(m, BL)],
                                             whh1[:, ((k * MT) + m) * 128:((k * MT) + m + 1) * 128],
                                             hT1[:, ts(k, BL)], start=False, stop=False)
                    for m in range(MT):
                        for k in range(4):
                            nc.tensor.matmul(g1[:, ts(m, BL)],
                                             wih1[:, ((k * MT) + m) * 128:((k * MT) + m + 1) * 128],
                                             hT0[:, ts(k, BL)], start=False,
                                             stop=(m == MT - 1 and k == 3))

                    sif1 = sw.tile([128, MT * BL], F32, name="sif1")
                    nc.scalar.activation(sif1[:], g1[:], AF.Sigmoid)
                    tg1 = sw.tile([128, QB], F32, name="tg1")
                    nc.scalar.activation(tg1[:], g1[:, 2 * QB:3 * QB], AF.Tanh)
                    t2 = sw.tile([128, QB], F32, name="t2")
                    nc.vector.tensor_mul(t2[:], sif1[:, 0:QB], tg1[:])
                    nc.vector.tensor_mul(c1[:], c1[:], sif1[:, QB:2 * QB])
                    nc.vector.tensor_add(c1[:], c1[:], t2[:])
                    tc1 = sw.tile([128, QB], F32, name="tc1")
                    nc.scalar.activation(tc1[:], c1[:], AF.Tanh)
                    nc.vector.tensor_mul(hT1[:], sif1[:, 3 * QB:4 * QB], tc1[:])

                    # head:  relu(W1h @ h1 + preh) -> logits
                    hps = sps.tile([128, 4 * BL], F32, name="hps")
                    nc.tensor.matmul(hps[:], idt[:], preh_t[:], start=True, stop=False)
                    for m in range(4):
                        for k in range(4):
                            nc.tensor.matmul(hps[:, ts(m, BL)],
                                             w1h[:, ((k * 4) + m) * 128:((k * 4) + m + 1) * 128],
                                             hT1[:, ts(k, BL)], start=False,
                                             stop=(m == 3 and k == 3))
                    relu = sw.tile([128, 4 * BL], F16, name="relu")
                    nc.scalar.activation(relu[:], hps[:], AF.Relu)

                    lg = sps.tile([C, BL], F32, name="lg")
                    for k in range(4):
                        nc.tensor.matmul(lg[:], w2[:, k * C:(k + 1) * C],
                                         relu[:, ts(k, BL)], start=(k == 0), stop=(k == 3))
                    logits = sw.tile([C, BL], F32, name="logits")
                    nc.vector.tensor_scalar_add(logits[:], lg[:], b2s[:])
                    nc.sync.dma_start(out=out_d[ds(i, 1), :, :].opt(), in_=logits[:])

                    # argmax -> onehot: batch-major logits via tiny matmuls
                    # (relu chunks as stationary), free-dim max, is_ge, then
                    # one PE transpose back to feature-major.
                    lgb = sps.tile([BL, C], F32, name="lgb")
                    for k in range(4):
                        nc.tensor.matmul(lgb[:], relu[:, ts(k, BL)],
                                         w2[:, k * C:(k + 1) * C],
                                         start=(k == 0), stop=False)
                    nc.tensor.matmul(lgb[:], onesrow[:], b2row[:],
                                     start=False, stop=True)
                    mx1 = sw.tile([BL, 1], F32, name="mx1")
                    nc.vector.tensor_reduce(mx1[:], lgb[:], mybir.AxisListType.X, mybir.AluOpType.max)
                    ohb = sw.tile([BL, C], F16, name="ohb")
                    nc.vector.tensor_scalar(ohb[:], lgb[:], mx1[:], None,
                                            mybir.AluOpType.is_ge)
                    ohp = sps.tile([C, BL], F16, name="ohp")
                    nc.tensor.transpose(ohp[:], ohb[:], idt[0:BL, 0:BL])
                    nc.vector.tensor_copy(onehot[:], ohp[:])

                with tc.For_i(0, (reps * t_steps) // unroll, staggered_reset=True) as i2:
                    for u in range(unroll):
                        tt = i2 * unroll + u
                        if reps != 1:
                            tt = nc.snap(tt % t_steps)
                        step(tt)

    nc.finalize()
    return nc


def _prep_maps(inputs: dict, t_steps: int):
    f32 = np.float32
    main_feats = np.asarray(inputs["main_feats"], f32)
    phys_feats = np.asarray(inputs["phys_feats"], f32)
    Wp = np.asarray(inputs["Wp"], f32)
    emb = np.asarray(inputs["emb"], f32)
    Wih0 = np.asarray(inputs["Wih0"], f32)
    Whh0 = np.asarray(inputs["Whh0"], f32)
    bih0 = np.asarray(inputs["bih0"], f32)
    bhh0 = np.asarray(inputs["bhh0"], f32)
    Wih1 = np.asarray(inputs["Wih1"], f32)
    Whh1 = np.asarray(inputs["Whh1"], f32)
    bih1 = np.asarray(inputs["bih1"], f32)
    bhh1 = np.asarray(inputs["bhh1"], f32)
    W1 = np.asarray(inputs["W1"], f32)
    b1 = np.asarray(inputs["b1"], f32)
    W2 = np.asarray(inputs["W2"], f32)
    b2 = np.asarray(inputs["b2"], f32)

    Wih0_m, Wih0_e = Wih0[:, :H], Wih0[:, H:]
    Mcomp = Wih0_m @ Wp                      # [G, D_IN]
    b0f = bih0 + bhh0                        # folded into P1 via aug row
    Mcomp_aug_T = np.concatenate([Mcomp, b0f[:, None]], axis=1).T  # [D_IN+1, G]
    McompT_v = _tile_stationary(Mcomp_aug_T[:256, :])
    McompB_v = Mcomp_aug_T[256:257, :].astype(np.float16)

    E0 = Wih0_e @ emb.T                      # [G, C]
    E0T_v = np.ascontiguousarray(E0.T).astype(np.float16)  # [C, G]

    b1f = bih1 + bhh1                        # [G]
    BIAS1_v = np.repeat(b1f.reshape(MT, 128).T[:, :, None], BL, axis=2).reshape(
        128, MT * BL).astype(np.float16)

    W1_h, W1_p = W1[:, :H], W1[:, H:]
    W1p_aug_T = np.concatenate([W1_p, b1[:, None]], axis=1).T   # [33, 512]
    W1pT_v = np.ascontiguousarray(W1p_aug_T).astype(np.float16)

    Whh0T_v = _tile_stationary(Whh0.T)
    Wih1T_v = _tile_stationary(Wih1.T)
    Whh1T_v = _tile_stationary(Whh1.T)
    W1hT_v = _tile_stationary(W1_h.T)
    W2T_v = (W2.T.reshape(4, 128, C).transpose(1, 0, 2).reshape(128, 4 * C)
             ).astype(np.float16)
    IDT_v = np.eye(128, dtype=np.float16)
    b2col_v = b2[:, None].astype(f32)

    b2r_v = b2[None, :].astype(np.float16)
    shared = dict(b2r=b2r_v, McompT=McompT_v, McompB=McompB_v, W1pT=W1pT_v, Whh0T=Whh0T_v,
                  Wih1T=Wih1T_v, Whh1T=Whh1T_v, W1hT=W1hT_v, W2T=W2T_v,
                  E0T=E0T_v, BIAS1=BIAS1_v, IDT=IDT_v, b2col=b2col_v)

    tok = t_steps * BL
    in_maps = []
    for j in range(NCORES):
        bsl = slice(j * BL, (j + 1) * BL)
        mj = main_feats[bsl, :t_steps]           # [BL, t, D_IN]
        mT = mj.transpose(2, 1, 0).reshape(D_IN, tok)          # col = t*BL + b
        mT = np.concatenate([mT, np.ones((1, tok), f32)], axis=0).astype(np.float16)
        pj = phys_feats[bsl, :t_steps]
        pT = pj.transpose(2, 1, 0).reshape(D_PHYS, tok)
        pT = np.concatenate([pT, np.ones((1, tok), f32)], axis=0).astype(np.float16)
        in_maps.append(dict(shared, mainT=np.ascontiguousarray(mT),
                            physT=np.ascontiguousarray(pT)))
    return in_maps


_BUILD_CACHE = {}


def get_built(t_steps: int, reps: int = 1):
    key = (t_steps, reps)
    if key not in _BUILD_CACHE:
        _BUILD_CACHE[key] = build(t_steps, reps)
    return _BUILD_CACHE[key]


def kernel(**inputs):
    t_steps = int(os.environ.get("ARLSTM_T", T))
    trace = bool(int(os.environ.get("ARLSTM_TRACE", "0")))
    nc = get_built(t_steps)
    in_maps = _prep_maps(inputs, t_steps)
    res = run_bass_kernel_spmd(nc, in_maps, core_ids=list(range(NCORES)),
                               trace=trace)
    kernel.last_result = res
    logits_full = np.empty((B, t_steps, C), np.float32)
    for j in range(NCORES):
        lg = res.results[j]["out_logits"]        # [t, C, BL]
        logits_full[j * BL:(j + 1) * BL] = lg.transpose(2, 0, 1)
    full = np.zeros((B, T, C), np.float32)
    full[:, :t_steps] = logits_full
    if t_steps != T:
        return full[:, :t_steps]
    attn = np.zeros((B, T), np.float32)
    return full, np.zeros_like(full), attn


# revision 16
# speedup vs baseline: 1.1922x; 1.1070x over previous
"""Trainium2 Bass kernel for the autoregressive 2-layer LSTM (nn_ArLSTM).

Strategy (phase A): data-parallel over batch. B=64 is sharded 8 ways (8
sequences per core); each core runs the full T=512 sequential scan locally
with no cross-core communication.

Algebraic restructuring vs the reference:
  - x_main @ Wih0_m is composed:  Wih0[:, :H] @ Wp  is one [4H, D_IN] matrix,
    so the per-step input term  pre0[t] = (Wih0_m@Wp) @ main_t + bih0 + bhh0
    is a parallel GEMM over all (b, t), done on-device before the scan.
  - emb[prev] enters only through Wih0[:, H:] @ emb[prev].  E0 = Wih0_e@emb.T
    is a [4H, 11] matrix; the per-step term is E0 @ onehot(prev), a K=11
    matmul.  onehot is built from the logits with a max-compare (no gather).
  - The head's phys term  W1[:, H:] @ ph_t + b1  is precomputed per (b, t).
  - All matmuls run in fp16 (1 cycle/row on PE) with fp32 PSUM accumulation;
    cell state c and all element-wise math stay fp32.  Measured end-to-end
    rel_err vs the fp32 reference ~6.5e-3 (argmax feedback is contractive).

Gate-bank layout: gates [4H, B_local] live in one PSUM bank [128, 16*BL]
where column-block m holds gate rows 128m..128m+127.  With PyTorch gate
order (i, f, g, o) the quarters are column ranges, so the whole LSTM cell
is element-wise ops on [128, 4*BL] slices.
"""

import os
import numpy as np

import concourse.bass as bass
import concourse.tile as tile
from concourse import bacc, mybir
from concourse.bass import ds, ts
from concourse.bass_utils import run_bass_kernel_spmd

F16 = mybir.dt.float16
F32 = mybir.dt.float32

B, T, D_IN, D_PHYS, H, C = 64, 512, 256, 32, 512, 11
NCORES = 8
BL = B // NCORES          # 8 sequences per core
G = 4 * H                 # 2048 gate rows
MT = G // 128             # 16 gate m-tiles
TOK = None                # set per build (t_steps * BL)


def _tile_stationary(wt: np.ndarray) -> np.ndarray:
    """[K, M] -> [128, (K/128)*(M/128)*128] fp16 stationary-tile layout.

    Free index = ((k_chunk*MT_loc + m_tile)*128 + col)."""
    K, M = wt.shape
    kc, mt = K // 128, M // 128
    return (
        wt.reshape(kc, 128, mt, 128).transpose(1, 0, 2, 3).reshape(128, kc * mt * 128)
    ).astype(np.float16)


def build(t_steps: int, reps: int = 1):
    tok = t_steps * BL
    nc = bacc.Bacc(None, target_bir_lowering=False, debug=False)

    # ---- DRAM parameters (per-core inputs) ----
    mainT = nc.declare_dram_parameter("mainT", [D_IN + 1, tok], F16, isOutput=False)    # aug ones row
    physT = nc.declare_dram_parameter("physT", [D_PHYS + 1, tok], F16, isOutput=False)  # aug ones row
    McompT = nc.declare_dram_parameter("McompT", [128, 2 * MT * 128], F16, isOutput=False)
    McompB = nc.declare_dram_parameter("McompB", [1, MT * 128], F16, isOutput=False)    # bias row
    W1pT = nc.declare_dram_parameter("W1pT", [D_PHYS + 1, 4 * 128], F16, isOutput=False)
    Whh0T = nc.declare_dram_parameter("Whh0T", [128, 4 * MT * 128], F16, isOutput=False)
    Wih1T = nc.declare_dram_parameter("Wih1T", [128, 4 * MT * 128], F16, isOutput=False)
    Whh1T = nc.declare_dram_parameter("Whh1T", [128, 4 * MT * 128], F16, isOutput=False)
    W1hT = nc.declare_dram_parameter("W1hT", [128, 4 * 4 * 128], F16, isOutput=False)
    W2T = nc.declare_dram_parameter("W2T", [128, 4 * C], F16, isOutput=False)
    E0T = nc.declare_dram_parameter("E0T", [C, MT * 128], F16, isOutput=False)
    BIAS1 = nc.declare_dram_parameter("BIAS1", [128, MT * BL], F16, isOutput=False)
    IDT = nc.declare_dram_parameter("IDT", [128, 128], F16, isOutput=False)
    b2col = nc.declare_dram_parameter("b2col", [C, 1], F32, isOutput=False)
    b2r = nc.declare_dram_parameter("b2r", [1, C], F16, isOutput=False)

    pre0_d = nc.dram_tensor("pre0_d", [128, t_steps, MT * BL], F16)
    preh_d = nc.dram_tensor("preh_d", [128, t_steps, 4 * BL], F16)
    out_d = nc.declare_dram_parameter("out_logits", [t_steps, C, BL], F32, isOutput=True)

    AF = mybir.ActivationFunctionType

    with tile.TileContext(nc) as tc:
        with tc.tile_pool(name="wpool", bufs=1) as wp:
            # persistent weights in SBUF
            whh0 = wp.tile([128, 4 * MT * 128], F16)
            wih1 = wp.tile([128, 4 * MT * 128], F16)
            whh1 = wp.tile([128, 4 * MT * 128], F16)
            w1h = wp.tile([128, 4 * 4 * 128], F16)
            w2 = wp.tile([128, 4 * C], F16)
            e0 = wp.tile([C, MT * 128], F16)
            bias1 = wp.tile([128, MT * BL], F16)
            idt = wp.tile([128, 128], F16)
            b2s = wp.tile([C, 1], F32)
            b2row = wp.tile([1, C], F16)
            onesrow = wp.tile([1, BL], F16)
            nc.sync.dma_start(out=whh0[:], in_=Whh0T[:])
            nc.sync.dma_start(out=wih1[:], in_=Wih1T[:])
            nc.sync.dma_start(out=whh1[:], in_=Whh1T[:])
            nc.sync.dma_start(out=w1h[:], in_=W1hT[:])
            nc.sync.dma_start(out=w2[:], in_=W2T[:])
            nc.sync.dma_start(out=e0[:], in_=E0T[:])
            nc.sync.dma_start(out=bias1[:], in_=BIAS1[:])
            nc.sync.dma_start(out=idt[:], in_=IDT[:])
            nc.sync.dma_start(out=b2s[:], in_=b2col[:])
            nc.sync.dma_start(out=b2row[:], in_=b2r[:])
            nc.vector.memset(onesrow[:], 1.0)

            # ---- P1/P2: parallel precompute GEMMs ----
            n_nc = tok // 512 if tok >= 512 else 1
            ncols = tok // n_nc
            with tc.tile_pool(name="p1pool", bufs=1) as pp, \
                 tc.tile_pool(name="p1work", bufs=3) as pw, \
                 tc.tile_pool(name="p1ps", bufs=4, space="PSUM") as pps:
                mcomp = pp.tile([128, 2 * MT * 128], F16)
                mcompb = pp.tile([1, MT * 128], F16)
                w1p = pp.tile([D_PHYS + 1, 4 * 128], F16)
                mainsb = pp.tile([128, 2 * tok], F16)
                mainsb1 = pp.tile([1, tok], F16)
                physsb = pp.tile([D_PHYS + 1, tok], F16)
                nc.sync.dma_start(out=mcomp[:], in_=McompT[:])
                nc.sync.dma_start(out=mcompb[:], in_=McompB[:])
                nc.sync.dma_start(out=w1p[:], in_=W1pT[:])
                nc.sync.dma_start(out=mainsb[:, 0:tok], in_=mainT[0:128, :])
                nc.sync.dma_start(out=mainsb[:, tok:2 * tok], in_=mainT[128:256, :])
                nc.sync.dma_start(out=mainsb1[:], in_=mainT[256:257, :])
                nc.sync.dma_start(out=physsb[:], in_=physT[:])

                for m in range(MT):
                    for n in range(n_nc):
                        ps = pps.tile([128, ncols], F32, name="ps")
                        cs = slice(n * ncols, (n + 1) * ncols)
                        nc.tensor.matmul(ps[:], mcomp[:, (0 * MT + m) * 128:(0 * MT + m + 1) * 128],
                                         mainsb[:, n * ncols:(n + 1) * ncols], start=True, stop=False)
                        nc.tensor.matmul(ps[:], mcomp[:, (1 * MT + m) * 128:(1 * MT + m + 1) * 128],
                                         mainsb[:, tok + n * ncols:tok + (n + 1) * ncols], start=False, stop=False)
                        nc.tensor.matmul(ps[:], mcompb[:, m * 128:(m + 1) * 128],
                                         mainsb1[:, cs], start=False, stop=True)
                        ev = pw.tile([128, ncols], F16, name="ev")
                        nc.scalar.activation(ev[:], ps[:], AF.Copy)
                        nc.sync.dma_start(
                            out=pre0_d[:, ds(n * (ncols // BL), ncols // BL), ts(m, BL)],
                            in_=ev[:].rearrange("p (t b) -> p t b", b=BL))
                for m in range(4):
                    for n in range(n_nc):
                        ps = pps.tile([128, ncols], F32, name="ps")
                        nc.tensor.matmul(ps[:], w1p[:, m * 128:(m + 1) * 128],
                                         physsb[:, n * ncols:(n + 1) * ncols], start=True, stop=True)
                        ev = pw.tile([128, ncols], F16, name="ev")
                        nc.scalar.activation(ev[:], ps[:], AF.Copy)
                        nc.sync.dma_start(
                            out=preh_d[:, ds(n * (ncols // BL), ncols // BL), ts(m, BL)],
                            in_=ev[:].rearrange("p (t b) -> p t b", b=BL))

            # ---- P4: the sequential scan ----
            with tc.tile_pool(name="state", bufs=1) as st, \
                 tc.tile_pool(name="sw", bufs=3) as sw, \
                 tc.tile_pool(name="sps", bufs=1, space="PSUM") as sps:
                hT0 = st.tile([128, 4 * BL], F16)
                hT1 = st.tile([128, 4 * BL], F16)
                c0 = st.tile([128, 4 * BL], F32)
                c1 = st.tile([128, 4 * BL], F32)
                onehot = st.tile([C, BL], F16)
                nc.vector.memset(hT0[:], 0.0)
                nc.vector.memset(hT1[:], 0.0)
                nc.vector.memset(c0[:], 0.0)
                nc.vector.memset(c1[:], 0.0)
                nc.vector.memset(onehot[:], 0.0)
                nc.vector.memset(onehot[0:1, :], 1.0)

                QB = 4 * BL  # quarter width in gate-bank columns (32)

                unroll = 1
                assert (reps * t_steps) % unroll == 0

                def step(i):
                    pre0_t = sw.tile([128, MT * BL], F16, name="pre0_t")
                    nc.sync.dma_start(out=pre0_t[:], in_=pre0_d[:, ds(i, 1), :].opt())
                    preh_t = sw.tile([128, 4 * BL], F16, name="preh_t")
                    nc.sync.dma_start(out=preh_t[:], in_=preh_d[:, ds(i, 1), :].opt())

                    # gates layer 0 (Whh0 first: no dependence on this
                    # step's DMA loads, so pre0_t/preh_t prefetch overlaps)
                    g0 = sps.tile([128, MT * BL], F32, name="g0")
                    for m in range(MT):
                        for k in range(4):
                            nc.tensor.matmul(g0[:, ts(m, BL)],
                                             whh0[:, ((k * MT) + m) * 128:((k * MT) + m + 1) * 128],
                                             hT0[:, ts(k, BL)], start=(m == 0 and k == 0), stop=False)
                    nc.tensor.matmul(g0[:], idt[:], pre0_t[:], start=False, stop=False)
                    for m in range(MT):
                        nc.tensor.matmul(g0[:, ts(m, BL)], e0[:, m * 128:(m + 1) * 128],
                                         onehot[:], start=False, stop=(m == MT - 1))

                    # cell 0 elementwise:  quarters i|f|g|o at col blocks
                    sif = sw.tile([128, MT * BL], F32, name="sif")
                    nc.scalar.activation(sif[:], g0[:], AF.Sigmoid)
                    tg0 = sw.tile([128, QB], F32, name="tg0")
                    nc.scalar.activation(tg0[:], g0[:, 2 * QB:3 * QB], AF.Tanh)
                    t1 = sw.tile([128, QB], F32, name="t1")
                    nc.vector.tensor_mul(t1[:], sif[:, 0:QB], tg0[:])
                    nc.vector.tensor_mul(c0[:], c0[:], sif[:, QB:2 * QB])
                    nc.vector.tensor_add(c0[:], c0[:], t1[:])
                    tc0 = sw.tile([128, QB], F32, name="tc0")
                    nc.scalar.activation(tc0[:], c0[:], AF.Tanh)
                    nc.vector.tensor_mul(hT0[:], sif[:, 3 * QB:4 * QB], tc0[:])

                    # gates layer 1
                    g1 = sps.tile([128, MT * BL], F32, name="g1")
                    nc.tensor.matmul(g1[:], idt[:], bias1[:], start=True, stop=False)
                    for m in range(MT):
                        for k in range(4):
                            nc.tensor.matmul(g1[:, ts(m, BL)],
                                             whh1[:, ((k * MT) + m) * 128:((k * MT) + m + 1) * 128],
                                             hT1[:, ts(k, BL)], start=False, stop=False)
                    for m in range(MT):
                        for k in range(4):
                            nc.tensor.matmul(g1[:, ts(m, BL)],
                                             wih1[:, ((k * MT) + m) * 128:((k * MT) + m + 1) * 128],
                                             hT0[:, ts(k, BL)], start=False,
                                             stop=(m == MT - 1 and k == 3))

                    sif1 = sw.tile([128, MT * BL], F32, name="sif1")
                    nc.scalar.activation(sif1[:], g1[:], AF.Sigmoid)
                    tg1 = sw.tile([128, QB], F32, name="tg1")
                    nc.scalar.activation(tg1[:], g1[:, 2 * QB:3 * QB], AF.Tanh)
                    t2 = sw.tile([128, QB], F32, name="t2")
                    nc.vector.tensor_mul(t2[:], sif1[:, 0:QB], tg1[:])
                    nc.vector.tensor_mul(c1[:], c1[:], sif1[:, QB:2 * QB])
                    nc.vector.tensor_add(c1[:], c1[:], t2[:])
                    tc1 = sw.tile([128, QB], F32, name="tc1")
                    nc.scalar.activation(tc1[:], c1[:], AF.Tanh)
                    nc.vector.tensor_mul(hT1[:], sif1[:, 3 * QB:4 * QB], tc1[:])

                    # head:  relu(W1h @ h1 + preh) -> logits
                    hps = sps.tile([128, 4 * BL], F32, name="hps")
                    nc.tensor.matmul(hps[:], idt[:], preh_t[:], start=True, stop=False)
                    for m in range(4):
                        for k in range(4):
                            nc.tensor.matmul(hps[:, ts(m, BL)],
                                             w1h[:, ((k * 4) + m) * 128:((k * 4) + m + 1) * 128],
                                             hT1[:, ts(k, BL)], start=False,
                                             stop=(m == 3 and k == 3))
                    relu = sw.tile([128, 4 * BL], F16, name="relu")
                    nc.scalar.activation(relu[:], hps[:], AF.Relu)

                    lg = sps.tile([C, BL], F32, name="lg")
                    for k in range(4):
                        nc.tensor.matmul(lg[:], w2[:, k * C:(k + 1) * C],
                                         relu[:, ts(k, BL)], start=(k == 0), stop=(k == 3))
                    logits = sw.tile([C, BL], F32, name="logits")
                    nc.vector.tensor_scalar_add(logits[:], lg[:], b2s[:])
                    nc.sync.dma_start(out=out_d[ds(i, 1), :, :].opt(), in_=logits[:])

                    # argmax -> onehot: batch-major logits via tiny matmuls
                    # (relu chunks as stationary), free-dim max, is_ge, then
                    # one PE transpose back to feature-major.
                    lgb = sps.tile([BL, C], F32, name="lgb")
                    for k in range(4):
                        nc.tensor.matmul(lgb[:], relu[:, ts(k, BL)],
                                         w2[:, k * C:(k + 1) * C],
                                         start=(k == 0), stop=False)
                    nc.tensor.matmul(lgb[:], onesrow[:], b2row[:],
                                     start=False, stop=True)
                    mx1 = sw.tile([BL, 1], F32, name="mx1")
                    nc.vector.tensor_reduce(mx1[:], lgb[:], mybir.AxisListType.X, mybir.AluOpType.max)
                    ohb = sw.tile([BL, C], F16, name="ohb")
                    nc.vector.tensor_scalar(ohb[:], lgb[:], mx1[:], None,
                                            mybir.AluOpType.is_ge)
                    ohp = sps.tile([C, BL], F16, name="ohp")
                    nc.tensor.transpose(ohp[:], ohb[:], idt[0:BL, 0:BL])
                    nc.vector.tensor_copy(onehot[:], ohp[:])

                with tc.For_i(0, (reps * t_steps) // unroll, staggered_reset=True) as i2:
                    for u in range(unroll):
                        tt = i2 * unroll + u
                        if reps != 1:
                            tt = nc.snap(tt % t_steps)
                        step(tt)

    nc.finalize()
    return nc


def _prep_maps(inputs: dict, t_steps: int):
    f32 = np.float32
    main_feats = np.asarray(inputs["main_feats"], f32)
    phys_feats = np.asarray(inputs["phys_feats"], f32)
    Wp = np.asarray(inputs["Wp"], f32)
    emb = np.asarray(inputs["emb"], f32)
    Wih0 = np.asarray(inputs["Wih0"], f32)
    Whh0 = np.asarray(inputs["Whh0"], f32)
    bih0 = np.asarray(inputs["bih0"], f32)
    bhh0 = np.asarray(inputs["bhh0"], f32)
    Wih1 = np.asarray(inputs["Wih1"], f32)
    Whh1 = np.asarray(inputs["Whh1"], f32)
    bih1 = np.asarray(inputs["bih1"], f32)
    bhh1 = np.asarray(inputs["bhh1"], f32)
    W1 = np.asarray(inputs["W1"], f32)
    b1 = np.asarray(inputs["b1"], f32)
    W2 = np.asarray(inputs["W2"], f32)
    b2 = np.asarray(inputs["b2"], f32)

    Wih0_m, Wih0_e = Wih0[:, :H], Wih0[:, H:]
    Mcomp = Wih0_m @ Wp                      # [G, D_IN]
    b0f = bih0 + bhh0                        # folded into P1 via aug row
    Mcomp_aug_T = np.concatenate([Mcomp, b0f[:, None]], axis=1).T  # [D_IN+1, G]
    McompT_v = _tile_stationary(Mcomp_aug_T[:256, :])
    McompB_v = Mcomp_aug_T[256:257, :].astype(np.float16)

    E0 = Wih0_e @ emb.T                      # [G, C]
    E0T_v = np.ascontiguousarray(E0.T).astype(np.float16)  # [C, G]

    b1f = bih1 + bhh1                        # [G]
    BIAS1_v = np.repeat(b1f.reshape(MT, 128).T[:, :, None], BL, axis=2).reshape(
        128, MT * BL).astype(np.float16)

    W1_h, W1_p = W1[:, :H], W1[:, H:]
    W1p_aug_T = np.concatenate([W1_p, b1[:, None]], axis=1).T   # [33, 512]
    W1pT_v = np.ascontiguousarray(W1p_aug_T).astype(np.float16)

    Whh0T_v = _tile_stationary(Whh0.T)
    Wih1T_v = _tile_stationary(Wih1.T)
    Whh1T_v = _tile_stationary(Whh1.T)
    W1hT_v = _tile_stationary(W1_h.T)
    W2T_v = (W2.T.reshape(4, 128, C).transpose(1, 0, 2).reshape(128, 4 * C)
             ).astype(np.float16)
    IDT_v = np.eye(128, dtype=np.float16)
    b2col_v = b2[:, None].astype(f32)

    b2r_v = b2[None, :].astype(np.float16)
    shared = dict(b2r=b2r_v, McompT=McompT_v, McompB=McompB_v, W1pT=W1pT_v, Whh0T=Whh0T_v,
                  Wih1T=Wih1T_v, Whh1T=Whh1T_v, W1hT=W1hT_v, W2T=W2T_v,
                  E0T=E0T_v, BIAS1=BIAS1_v, IDT=IDT_v, b2col=b2col_v)

    tok = t_steps * BL
    in_maps = []
    for j in range(NCORES):
        bsl = slice(j * BL, (j + 1) * BL)
        mj = main_feats[bsl, :t_steps]           # [BL, t, D_IN]
        mT = mj.transpose(2, 1, 0).reshape(D_IN, tok)          # col = t*BL + b
        mT = np.concatenate([mT, np.ones((1, tok), f32)], axis=0).astype(np.float16)
        pj = phys_feats[bsl, :t_steps]
        pT = pj.transpose(2, 1, 0).reshape(D_PHYS, tok)
        pT = np.concatenate([pT, np.ones((1, tok), f32)], axis=0).astype(np.float16)
        in_maps.append(dict(shared, mainT=np.ascontiguousarray(mT),
                            physT=np.ascontiguousarray(pT)))
    return in_maps


_BUILD_CACHE = {}


def get_built(t_steps: int, reps: int = 1):
    key = (t_steps, reps)
    if key not in _BUILD_CACHE:
        _BUILD_CACHE[key] = build(t_steps, reps)
    return _BUILD_CACHE[key]


def kernel(**inputs):
    t_steps = int(os.environ.get("ARLSTM_T", T))
    trace = bool(int(os.environ.get("ARLSTM_TRACE", "0")))
    nc = get_built(t_steps)
    in_maps = _prep_maps(inputs, t_steps)
    res = run_bass_kernel_spmd(nc, in_maps, core_ids=list(range(NCORES)),
                               trace=trace)
    kernel.last_result = res
    logits_full = np.empty((B, t_steps, C), np.float32)
    for j in range(NCORES):
        lg = res.results[j]["out_logits"]        # [t, C, BL]
        logits_full[j * BL:(j + 1) * BL] = lg.transpose(2, 0, 1)
    full = np.zeros((B, T, C), np.float32)
    full[:, :t_steps] = logits_full
    if t_steps != T:
        return full[:, :t_steps]
    attn = np.zeros((B, T), np.float32)
    return full, np.zeros_like(full), attn
